# revision 1
# baseline (speedup 1.0000x reference)
"""Trainium2 Bass kernel for CachedRoPEAttention.

Sharding: 8 cores = batch(2) x head-groups(4). Each core computes 4 heads of
one batch element end-to-end (q/k/v proj in [e,t] layout, RoPE, causal
flash-style attention with ones-row softmax denominators, out_proj partial),
host sums the 4 tensor-parallel partials per batch.

All matmuls run in float32r (fp32 bits, HW rounds stream to ~12-bit mantissa,
1 cycle/row at N>=256).
"""
import sys
sys.path.insert(0, "/opt/trn_rl_repo")

import numpy as np

import concourse.bass as bass
import concourse.bacc as bacc
import concourse.mybir as mybir
import concourse.tile as tile
from concourse.bass_utils import run_bass_kernel_spmd

F32 = mybir.dt.float32
F32R = mybir.dt.float32r

D, H, DH, T, B = 1024, 16, 64, 2048, 2
HG, HPC, EC = 4, 4, 256      # head groups, heads/core, e-width/core
KT = D // 128                # 8 contraction tiles over d_model
PT = EC // 128               # 2 e-partition-tiles per core
NB = T // 512                # 4 t-blocks
NTT = T // 128               # 16 t-tiles

_NC_CACHE = {}


def _build_nc():
    nc = bacc.Bacc(None, target_bir_lowering=False)

    xT_d = nc.dram_tensor("xT", [D, T], F32R, kind="ExternalInput")
    wqT_d = nc.dram_tensor("wqT", [D, EC], F32R, kind="ExternalInput")
    wkT_d = nc.dram_tensor("wkT", [D, EC], F32R, kind="ExternalInput")
    wvT_d = nc.dram_tensor("wvT", [D, EC], F32R, kind="ExternalInput")
    woT_d = nc.dram_tensor("woT", [EC, D], F32R, kind="ExternalInput")
    cos2_d = nc.dram_tensor("cos2", [128, T], F32, kind="ExternalInput")
    sin2p_d = nc.dram_tensor("sin2p", [128, T], F32, kind="ExternalInput")
    tri_d = nc.dram_tensor("tri", [128, 128], F32R, kind="ExternalInput")
    ztri_d = nc.dram_tensor("ztri", [128, 256], F32R, kind="ExternalInput")
    outT_d = nc.dram_tensor("outT", [D, T], F32, kind="ExternalOutput")

    with tile.TileContext(nc) as tc:
        with tc.tile_pool(name="perm", bufs=1) as perm, \
             tc.tile_pool(name="psum", bufs=1, space="PSUM") as psp, \
             tc.tile_pool(name="dram", bufs=2, space="DRAM") as drp:
            # ---- persistent tiles
            qT = perm.tile([128, PT, T], F32R)
            kT = perm.tile([128, PT, T], F32R)
            v_sb = perm.tile([128, NTT, HPC, 65], F32R)
            wo_sb = perm.tile([128, 2, D], F32R)
            tri_sb = perm.tile([128, 128], F32R)
            ztri_sb = perm.tile([128, 256], F32R)
            ones_sb = perm.tile([33, 64], F32R)
            # ================= phase 1: projections + RoPE =================
            with tc.tile_pool(name="ph1", bufs=1) as ph1, \
                 tc.tile_pool(name="rw", bufs=3) as rw:
                x_sb = ph1.tile([128, KT, T], F32R)
                wq_sb = ph1.tile([128, KT, EC], F32R)
                wk_sb = ph1.tile([128, KT, EC], F32R)
                wv_sb = ph1.tile([128, KT, EC], F32R)
                cos_sb = ph1.tile([128, T], F32)
                sin_sb = ph1.tile([128, T], F32)
                # load order: small consts first (attention phase depends on
                # tri/ztri/cos/sin — tail-loading them stalls phase 2), then
                # x on the sync queue, weights on the scalar queue in
                # consumption order (wv first).
                nc.scalar.dma_start(out=cos_sb, in_=cos2_d.ap())
                nc.scalar.dma_start(out=sin_sb, in_=sin2p_d.ap())
                nc.scalar.dma_start(out=tri_sb, in_=tri_d.ap())
                nc.scalar.dma_start(out=ztri_sb, in_=ztri_d.ap())
                # wv first (v proj is the first consumer), then x split into
                # t-halves across both queues so the first 8 v-proj t-tiles
                # can start after half the x traffic instead of all of it
                for k in range(KT):
                    r = slice(128 * k, 128 * k + 128)
                    nc.scalar.dma_start(out=wv_sb[:, k, :], in_=wvT_d.ap()[r, :])
                for half in range(2):
                    cols = slice(1024 * half, 1024 * half + 1024)
                    for k in range(KT):
                        r = slice(128 * k, 128 * k + 128)
                        q = nc.sync if k % 2 == 0 else nc.scalar
                        q.dma_start(out=x_sb[:, k, cols],
                                    in_=xT_d.ap()[r, cols])
                for k in range(KT):
                    r = slice(128 * k, 128 * k + 128)
                    nc.sync.dma_start(out=wq_sb[:, k, :], in_=wqT_d.ap()[r, :])
                    nc.scalar.dma_start(out=wk_sb[:, k, :], in_=wkT_d.ap()[r, :])
                for ct in range(2):
                    nc.sync.dma_start(out=wo_sb[:, ct, :],
                                      in_=woT_d.ap()[128 * ct:128 * ct + 128, :])
                # ones column of v (attn denominators) + ones rows for the
                # reciprocal-broadcast matmuls: memset an fp32 scratch, then
                # DVE broadcast-copies (the old strided DMA broadcast
                # exploded into ~8k 4-byte descriptors; direct f32r memset
                # fails walrus codegen)
                ones1 = ph1.tile([128, 1], F32)
                nc.vector.memset(ones1, 1.0)
                nc.vector.tensor_copy(
                    out=v_sb[:, :, :, 64:65].rearrange("p a b c -> p (a b c)"),
                    in_=ones1[:, 0:1].broadcast_to((128, NTT * HPC)))
                nc.vector.tensor_copy(out=ones_sb[0:1, :],
                                      in_=ones1[0:1, 0:1].broadcast_to((1, 64)))
                nc.vector.tensor_copy(out=ones_sb[32:33, :],
                                      in_=ones1[32:33, 0:1].broadcast_to((1, 64)))

                # v projection first (attention needs v earliest)
                for tt in range(NTT):
                    ps = psp.tile([128, 256], F32, tag="mm", bufs=2, name=f"psv{tt}")
                    for k in range(KT):
                        nc.tensor.matmul(
                            ps, x_sb[:, k, 128 * tt:128 * tt + 128],
                            wv_sb[:, k, :],
                            start=(k == 0), stop=(k == KT - 1))
                    nc.vector.tensor_copy(
                        out=v_sb[:, tt, :, 0:64],
                        in_=ps.rearrange("p (h d) -> p h d", h=HPC))

                # q/k projections + fused RoPE, chunk = [128, 512];
                # pair-0 chunks first so attention can start early
                for pt in range(PT):
                    for nb in range(NB):
                        cols = slice(512 * nb, 512 * nb + 512)
                        for w_sb, dst, wnm in ((wq_sb, qT, "q"), (wk_sb, kT, "k")):
                            ps = psp.tile([128, 512], F32, tag="mm", bufs=2,
                                          name=f"ps{wnm}{pt}{nb}")
                            for k in range(KT):
                                nc.tensor.matmul(
                                    ps, w_sb[:, k, 128 * pt:128 * pt + 128],
                                    x_sb[:, k, cols],
                                    start=(k == 0), stop=(k == KT - 1))
                            qc = rw.tile([128, 512], F32R, tag="qc")
                            nc.vector.tensor_mul(out=qc, in0=ps, in1=cos_sb[:, cols])
                            # sin term: multiply by pre-shuffled sin const,
                            # THEN partition-shuffle the SBUF product via DMA
                            qp = rw.tile([128, 512], F32R, tag="qp")
                            nc.vector.tensor_mul(out=qp, in0=ps, in1=sin_sb[:, cols])
                            shuf = rw.tile([128, 512], F32R, tag="shuf")
                            for b2 in range(2):
                                base = 64 * b2
                                nc.sync.dma_start(out=shuf[base:base + 32, :],
                                                  in_=qp[base + 32:base + 64, :])
                                nc.sync.dma_start(out=shuf[base + 32:base + 64, :],
                                                  in_=qp[base:base + 32, :])
                            nc.vector.tensor_add(out=dst[:, pt, cols], in0=qc, in1=shuf)

            # ================= phase 2: attention + out_proj =================
            with tc.tile_pool(name="att", bufs=1) as att, \
                 tc.tile_pool(name="ew", bufs=3) as ew:
                OT_all = att.tile([128, PT, T], F32R)
                for blk in range(NB):
                    cols = slice(512 * blk, 512 * blk + 512)
                    for pair in range(PT):
                        prow = slice(0, 128)
                        ot = [psp.tile([65, 512], F32, tag="ot", bufs=2,
                                       name=f"ot{blk}{pair}{hd}")
                              for hd in range(2)]
                        ntk = 2 * (blk + 1)
                        last = (ntk - 1, 1)
                        for tkp in range(ntk):
                            t0 = 2 * tkp
                            lo = [max(0, 128 * (t0 + h - 4 * blk)) for h in (0, 1)]
                            lop = lo[0]
                            st = [psp.tile([128, 2, 512], F32, tag="st2", bufs=2,
                                           name=f"st{blk}{pair}{tkp}{hd}")
                                  for hd in range(2)]
                            # h outer / hd inner: consecutive matmuls hit
                            # alternating 64-row groups of the PE array, so
                            # the two heads' K=64 matmuls run concurrently
                            for h in (0, 1):
                                tt = t0 + h
                                for hd in range(2):
                                    hrow = slice(64 * hd, 64 * hd + 64)
                                    nc.tensor.matmul(
                                        st[hd][:, h, lop:512],
                                        kT[hrow, pair, 128 * tt:128 * tt + 128],
                                        qT[hrow, pair, 512 * blk + lop:512 * blk + 512],
                                        start=True, stop=True)
                            ex = [ew.tile([128, 2, 512], F32R, tag="ex",
                                          name=f"ex{blk}{pair}{tkp}{hd2}")
                                  for hd2 in range(2)]
                            for hd in range(2):
                                nc.scalar.activation(
                                    out=ex[hd][:, :, lop:512],
                                    in_=st[hd][:, :, lop:512],
                                    func=mybir.ActivationFunctionType.Exp,
                                    scale=0.125)
                                # causal masks on diagonal tiles
                                for h in (0, 1):
                                    j = t0 + h - 4 * blk
                                    if j < 0:
                                        continue
                                    lo_h = lo[h]
                                    if lo_h == lop:
                                        nc.vector.tensor_mul(
                                            out=ex[hd][:, h, lo_h:lo_h + 128],
                                            in0=ex[hd][:, h, lo_h:lo_h + 128],
                                            in1=tri_sb)
                                    else:
                                        w = lo_h + 128 - lop
                                        nc.vector.tensor_mul(
                                            out=ex[hd][:, h, lop:lo_h + 128],
                                            in0=ex[hd][:, h, lop:lo_h + 128],
                                            in1=ztri_sb[:, 0:w])
                                for h in (0, 1):
                                    tt = t0 + h
                                    lo_h = lo[h]
                                    nc.tensor.matmul(
                                        ot[hd][:, lo_h:512],
                                        v_sb[:, tt, 2 * pair + hd, :],
                                        ex[hd][:, h, lo_h:512],
                                        start=(tkp == 0 and h == 0),
                                        stop=(tkp, h) == last)
                        # copy raw OT out of PSUM promptly (frees the banks),
                        # then denominators -> DRAM-bounce broadcast -> normalize
                        ots = [ew.tile([65, 512], F32, tag="ots", bufs=2,
                                       name=f"ots{blk}{pair}{hd}")
                               for hd in range(2)]
                        for hd in range(2):
                            nc.vector.tensor_copy(out=ots[hd], in_=ot[hd])
                        # fast approx reciprocal (~51 ULP, 5x faster than
                        # InstReciprocal) -> PE ones-outer-product broadcast
                        # to 64 partitions in PSUM -> normalize muls read it
                        # gather both denominator rows at base partition 0
                        # (reciprocal_approx_fast silently corrupts at any
                        # nonzero base partition), one approx recip over
                        # both, then an f32r-rounding copy for the PE
                        # broadcast matmuls
                        den = ew.tile([1, 2, 512], F32, tag="rc2",
                                      name=f"rc{blk}{pair}")
                        nc.vector.tensor_copy(out=den[0:1, 0, :],
                                              in_=ots[0][64:65, :])
                        nc.vector.tensor_copy(out=den[0:1, 1, :],
                                              in_=ots[1][64:65, :])
                        rc2 = ew.tile([1, 2, 512], F32, tag="rc2b",
                                      name=f"rcb{blk}{pair}")
                        nc.vector.reciprocal_approx_fast(
                            out=rc2.rearrange("p a b -> p (a b)"),
                            in_=den.rearrange("p a b -> p (a b)"))
                        rc2r = ew.tile([1, 2, 512], F32R, tag="rc2r",
                                       name=f"rcr{blk}{pair}")
                        nc.vector.tensor_copy(
                            out=rc2r.rearrange("p a b -> p (a b)"),
                            in_=rc2.rearrange("p a b -> p (a b)"))
                        rcp = [psp.tile([64, 512], F32, tag="ot", bufs=2,
                                        name=f"rcp{blk}{pair}{hd}")
                               for hd in range(2)]
                        nc.tensor.matmul(rcp[0], ones_sb[0:1, :],
                                         rc2r[0:1, 0, :],
                                         start=True, stop=True)
                        nc.tensor.matmul(rcp[1], ones_sb[0:1, :],
                                         rc2r[0:1, 1, :],
                                         start=True, stop=True)
                        nc.vector.tensor_mul(out=OT_all[0:64, pair, cols],
                                             in0=ots[0][0:64, :], in1=rcp[0])
                        nc.vector.tensor_mul(out=OT_all[64:128, pair, cols],
                                             in0=ots[1][0:64, :], in1=rcp[1])
                    # out_proj for this t-block
                    for m in range(KT):
                        fp = psp.tile([128, 512], F32, tag="mm", bufs=2,
                                      name=f"fp{blk}{m}")
                        for ct in range(2):
                            nc.tensor.matmul(
                                fp, wo_sb[:, ct, 128 * m:128 * m + 128],
                                OT_all[:, ct, cols],
                                start=(ct == 0), stop=(ct == 1))
                        fs = ew.tile([128, 512], F32, tag="fs",
                                     name=f"fs{blk}{m}")
                        nc.vector.tensor_copy(out=fs, in_=fp)
                        nc.sync.dma_start(
                            out=outT_d.ap()[128 * m:128 * m + 128, cols], in_=fs)

    nc.compile()
    return nc


def _consts():
    i = np.arange(32)
    theta = 1.0 / (10000.0 ** (2.0 * i / 64))
    ang = np.outer(np.arange(T, dtype=np.float64), theta)
    p = np.arange(128)
    cos2 = np.cos(ang[:, p % 32]).T.astype(np.float32)
    sgn = np.where((p % 64) < 32, -1.0, 1.0)
    sin2s = (np.sin(ang[:, p % 32]) * sgn).T.astype(np.float32)
    cos2 = np.ascontiguousarray(cos2)
    # pre-shuffled sin so the kernel can multiply BEFORE the partition shuffle:
    # shuf(q * sin2p)[p] = q[p^32] * sin2s[p]
    sin2p = np.ascontiguousarray(sin2s[p ^ 32])
    r, c = np.meshgrid(np.arange(128), np.arange(128), indexing="ij")
    tri = (r <= c).astype(np.float32)
    ztri = np.ascontiguousarray(
        np.concatenate([np.zeros((128, 128), np.float32), tri], axis=1))
    return cos2, sin2p, tri, ztri


def kernel(x, Wq, Wk, Wv, Wo, _trace=False):
    x = np.asarray(x, dtype=np.float32)
    Wq = np.asarray(Wq, dtype=np.float32)
    Wk = np.asarray(Wk, dtype=np.float32)
    Wv = np.asarray(Wv, dtype=np.float32)
    Wo = np.asarray(Wo, dtype=np.float32)

    if "nc" not in _NC_CACHE:
        _NC_CACHE["nc"] = _build_nc()
    nc = _NC_CACHE["nc"]

    cos2, sin2p, tri, ztri = _consts()
    xTs = [np.ascontiguousarray(x[b].T) for b in range(B)]
    WqT, WkT, WvT, WoT = Wq.T, Wk.T, Wv.T, Wo.T

    in_maps = []
    for c in range(8):
        b, g = c // HG, c % HG
        cs = slice(EC * g, EC * g + EC)
        in_maps.append({
            "xT": xTs[b],
            "wqT": np.ascontiguousarray(WqT[:, cs]),
            "wkT": np.ascontiguousarray(WkT[:, cs]),
            "wvT": np.ascontiguousarray(WvT[:, cs]),
            "woT": np.ascontiguousarray(WoT[cs, :]),
            "cos2": cos2, "sin2p": sin2p,
            "tri": tri, "ztri": ztri,
        })

    kw = {}
    if _trace:
        kw = dict(trace=True, trace_cores=list(range(8)))
    res = run_bass_kernel_spmd(nc, in_maps, core_ids=list(range(8)), **kw)

    out = np.zeros((B, T, D), np.float32)
    for c in range(8):
        out[c // HG] += res.results[c]["outT"].T
    if _trace:
        return out, res
    return out



# revision 2
# speedup vs baseline: 1.3284x; 1.3284x over previous
"""Trainium2 Bass kernel for CachedRoPEAttention.

Sharding: 8 cores = batch(2) x head-groups(4). Each core computes 4 heads of
one batch element end-to-end (q/k/v proj, RoPE, causal attention with
ones-row softmax denominators, out_proj partial); host sums the 4
tensor-parallel partials per batch.

v2: bf16 end-to-end (fp32 PSUM accumulation). Halves HBM traffic and DVE
cycles vs the f32r baseline; matmul stream rate is unchanged but the
schedule is restructured to keep the PE dense (HAM stays warm):
 - projections: stationary weights, 4 MMs per LDWEIGHTS, N=512
 - attention: per-key-tile software pipeline (scores -> exp -> mask -> PV)
   with PSUM budget mm(2) + st(4) + ot(2) = 8 banks
 - out_proj for block b is emitted one attention block late so its matmuls
   fill PE gaps in the ACT-limited attention tail.
"""
import sys
sys.path.insert(0, "/opt/trn_rl_repo")

import numpy as np
import ml_dtypes

import concourse.bass as bass
import concourse.bacc as bacc
import concourse.mybir as mybir
import concourse.tile as tile
from concourse.bass_utils import run_bass_kernel_spmd

F32 = mybir.dt.float32
F32R = mybir.dt.float32r
BF16 = mybir.dt.bfloat16
NPBF16 = ml_dtypes.bfloat16

D, H, DH, T, B = 1024, 16, 64, 2048, 2
HG, HPC, EC = 4, 4, 256      # head groups, heads/core, e-width/core
KT = D // 128                # 8 contraction tiles over d_model
NB = T // 512                # 4 t-blocks
NTT = T // 128               # 16 t-tiles

_NC_CACHE = {}


def _build_nc():
    nc = bacc.Bacc(None, target_bir_lowering=False)

    xT_d = nc.dram_tensor("xT", [D, T], BF16, kind="ExternalInput")
    wqT_d = nc.dram_tensor("wqT", [D, EC], BF16, kind="ExternalInput")
    wkT_d = nc.dram_tensor("wkT", [D, EC], BF16, kind="ExternalInput")
    wvT_d = nc.dram_tensor("wvT", [D, EC], BF16, kind="ExternalInput")
    woT_d = nc.dram_tensor("woT", [EC, D], BF16, kind="ExternalInput")
    cos2_d = nc.dram_tensor("cos2", [128, T], BF16, kind="ExternalInput")
    sin2p_d = nc.dram_tensor("sin2p", [128, T], BF16, kind="ExternalInput")
    tri2_d = nc.dram_tensor("tri2", [128, 256], BF16, kind="ExternalInput")
    outT_d = nc.dram_tensor("outT", [D, T], BF16, kind="ExternalOutput")

    with tile.TileContext(nc) as tc:
        with tc.tile_pool(name="perm", bufs=1) as perm, \
             tc.tile_pool(name="psum", bufs=1, space="PSUM") as psp, \
             tc.tile_pool(name="rw", bufs=3) as rw, \
             tc.tile_pool(name="ew", bufs=3) as ew:
            # ---- persistent tiles
            x_sb = perm.tile([128, KT, T], BF16)
            wq_sb = perm.tile([128, KT, EC], BF16)
            wk_sb = perm.tile([128, KT, EC], BF16)
            wv_sb = perm.tile([128, KT, EC], BF16)
            wo_sb = perm.tile([128, 2, D], BF16)
            cos_sb = perm.tile([128, T], BF16)
            sin_sb = perm.tile([128, T], BF16)
            tri_sb = perm.tile([128, 2, 128], BF16)
            qT = perm.tile([128, 2, T], BF16)
            kT = perm.tile([128, 2, T], BF16)
            v_sb = perm.tile([128, NTT, HPC, 65], BF16)
            OT_all = perm.tile([128, 2, T], BF16)
            ones_sb = perm.tile([1, 64], F32R)

            # ---- input DMAs, in consumption order.
            # scalar queue: wv, tri, x-half0-odd-k, wq, x-half1-odd-k, wo
            # sync queue:   x-half0-even-k, cos, sin, wk, x-half1-even-k
            for k in range(KT):
                r = slice(128 * k, 128 * k + 128)
                nc.scalar.dma_start(out=wv_sb[:, k, :], in_=wvT_d.ap()[r, :])
            nc.scalar.dma_start(out=tri_sb.rearrange("p a b -> p (a b)"),
                                in_=tri2_d.ap())
            for k in range(0, KT, 2):
                r = slice(128 * k, 128 * k + 128)
                nc.sync.dma_start(out=x_sb[:, k, 0:1024],
                                  in_=xT_d.ap()[r, 0:1024])
            for k in range(1, KT, 2):
                r = slice(128 * k, 128 * k + 128)
                nc.scalar.dma_start(out=x_sb[:, k, 0:1024],
                                    in_=xT_d.ap()[r, 0:1024])
            nc.sync.dma_start(out=cos_sb, in_=cos2_d.ap())
            nc.sync.dma_start(out=sin_sb, in_=sin2p_d.ap())
            for k in range(KT):
                r = slice(128 * k, 128 * k + 128)
                nc.scalar.dma_start(out=wq_sb[:, k, :], in_=wqT_d.ap()[r, :])
                nc.sync.dma_start(out=wk_sb[:, k, :], in_=wkT_d.ap()[r, :])
            for k in range(0, KT, 2):
                r = slice(128 * k, 128 * k + 128)
                nc.sync.dma_start(out=x_sb[:, k, 1024:2048],
                                  in_=xT_d.ap()[r, 1024:2048])
            for k in range(1, KT, 2):
                r = slice(128 * k, 128 * k + 128)
                nc.scalar.dma_start(out=x_sb[:, k, 1024:2048],
                                    in_=xT_d.ap()[r, 1024:2048])
            for ct in range(2):
                nc.scalar.dma_start(out=wo_sb[:, ct, :],
                                    in_=woT_d.ap()[128 * ct:128 * ct + 128, :])

            # ones: v denominator column (bf16) + f32r ones row for the
            # reciprocal-broadcast matmuls
            ones1 = rw.tile([128, 1], F32, tag="ones1", bufs=1)
            nc.vector.memset(ones1, 1.0)
            nc.vector.tensor_copy(
                out=v_sb[:, :, :, 64:65].rearrange("p a b c -> p (a b c)"),
                in_=ones1[:, 0:1].broadcast_to((128, NTT * HPC)))
            nc.vector.tensor_copy(out=ones_sb,
                                  in_=ones1[0:1, 0:1].broadcast_to((1, 64)))

            # ---------------- helpers ----------------
            def v_proj(tt_lo, tt_hi):
                # v[t, e] for t-tiles [tt_lo, tt_hi): stationary x tile,
                # moving wv (N=256)
                for tt in range(tt_lo, tt_hi):
                    ps = psp.tile([128, 512], F32, tag="mm", bufs=2,
                                  name=f"psv{tt}")
                    for k in range(KT):
                        nc.tensor.matmul(
                            ps[:, 0:256],
                            x_sb[:, k, 128 * tt:128 * tt + 128],
                            wv_sb[:, k, :],
                            start=(k == 0), stop=(k == KT - 1))
                    nc.vector.tensor_copy(
                        out=v_sb[:, tt, :, 0:64],
                        in_=ps[:, 0:256].rearrange("p (h d) -> p h d", h=HPC))

            def qk_proj_half(w_sb, dst, pt, half, wnm):
                # two 512-col t-blocks (nb = 2*half, 2*half+1) of q or k for
                # partition-tile pt: stationary w chunk, 2 MMs per LDW,
                # fp32 PSUM accumulate over k, then fused RoPE per block.
                ps = psp.tile([128, 2, 512], F32, tag="st", bufs=2,
                              name=f"ps{wnm}{pt}{half}")
                for k in range(KT):
                    for j in range(2):
                        cols = slice(512 * (2 * half + j),
                                     512 * (2 * half + j) + 512)
                        nc.tensor.matmul(
                            ps[:, j, :],
                            w_sb[:, k, 128 * pt:128 * pt + 128],
                            x_sb[:, k, cols],
                            start=(k == 0), stop=(k == KT - 1))
                for j in range(2):
                    nb = 2 * half + j
                    cols = slice(512 * nb, 512 * nb + 512)
                    qc = rw.tile([128, 512], BF16, tag="qc")
                    nc.vector.tensor_mul(out=qc, in0=ps[:, j, :],
                                         in1=cos_sb[:, cols])
                    # sin term: multiply by pre-shuffled sin const, THEN
                    # partition-shuffle (p ^ 32) the product via DMA
                    qp = rw.tile([128, 512], BF16, tag="qp")
                    nc.vector.tensor_mul(out=qp, in0=ps[:, j, :],
                                         in1=sin_sb[:, cols])
                    shuf = rw.tile([128, 512], BF16, tag="shuf")
                    for b2 in range(2):
                        base = 64 * b2
                        q_ = nc.sync if b2 == 0 else nc.scalar
                        q_.dma_start(out=shuf[base:base + 32, :],
                                     in_=qp[base + 32:base + 64, :])
                        q_.dma_start(out=shuf[base + 32:base + 64, :],
                                     in_=qp[base:base + 32, :])
                    nc.vector.tensor_add(out=dst[:, pt, cols],
                                         in0=qc, in1=shuf)

            def attention(blk, pair):
                # causal attention for q-block blk (512 queries), heads
                # 2*pair + {0,1}; per-key-tile pipeline.
                qcols = slice(512 * blk, 512 * blk + 512)
                nkt = 4 * (blk + 1)
                ot = psp.tile([65, 2, 512], F32, tag="ot", bufs=1,
                              name=f"ot{blk}{pair}")
                for kt in range(nkt):
                    lop = max(0, 128 * kt - 512 * blk)
                    st = psp.tile([128, 2, 512], F32, tag="st", bufs=2,
                                  name=f"st{blk}{pair}{kt}")
                    for hd in range(2):
                        hrow = slice(64 * hd, 64 * hd + 64)
                        nc.tensor.matmul(
                            st[:, hd, lop:512],
                            kT[hrow, pair, 128 * kt:128 * kt + 128],
                            qT[hrow, pair, 512 * blk + lop:512 * blk + 512],
                            start=True, stop=True)
                    ex = ew.tile([128, 2, 512], BF16, tag="ex", bufs=3,
                                 name=f"ex{blk}{pair}{kt}")
                    nc.scalar.activation(
                        out=ex[:, :, lop:512], in_=st[:, :, lop:512],
                        func=mybir.ActivationFunctionType.Exp, scale=0.125)
                    if kt >= 4 * blk:  # diagonal tile: causal mask
                        nc.vector.tensor_mul(
                            out=ex[:, :, lop:lop + 128],
                            in0=ex[:, :, lop:lop + 128],
                            in1=tri_sb)
                    for hd in range(2):
                        nc.tensor.matmul(
                            ot[:, hd, lop:512],
                            v_sb[:, kt, 2 * pair + hd, :],
                            ex[:, hd, lop:512],
                            start=(kt == 0), stop=(kt == nkt - 1))
                # softmax denominators: ones row (partition 64 of ot) ->
                # f32r copy -> PE broadcast to 64 partitions -> approx
                # reciprocal -> normalize muls into OT_all
                den = ew.tile([1, 2, 512], F32R, tag="den",
                              name=f"den{blk}{pair}")
                nc.vector.tensor_copy(out=den.rearrange("p a b -> p (a b)"),
                                      in_=ot[64:65, :, :].rearrange(
                                          "p a b -> p (a b)"))
                dbc = psp.tile([64, 2, 512], F32, tag="st", bufs=2,
                               name=f"dbc{blk}{pair}")
                for hd in range(2):
                    nc.tensor.matmul(dbc[:, hd, :], ones_sb, den[0:1, hd, :],
                                     start=True, stop=True)
                rc = ew.tile([64, 2, 512], F32, tag="rc",
                             name=f"rc{blk}{pair}")
                nc.vector.reciprocal_approx_fast(
                    out=rc.rearrange("p a b -> p (a b)"),
                    in_=dbc.rearrange("p a b -> p (a b)"))
                for hd in range(2):
                    nc.vector.tensor_mul(
                        out=OT_all[64 * hd:64 * hd + 64, pair, qcols],
                        in0=ot[0:64, hd, :], in1=rc[:, hd, :])

            def out_proj(blk):
                cols = slice(512 * blk, 512 * blk + 512)
                for m in range(KT):
                    fp = psp.tile([128, 512], F32, tag="mm", bufs=2,
                                  name=f"fp{blk}{m}")
                    for ct in range(2):
                        nc.tensor.matmul(
                            fp, wo_sb[:, ct, 128 * m:128 * m + 128],
                            OT_all[:, ct, cols],
                            start=(ct == 0), stop=(ct == 1))
                    fs = ew.tile([128, 512], BF16, tag="fs",
                                 name=f"fs{blk}{m}")
                    nc.vector.tensor_copy(out=fs, in_=fp)
                    q_ = nc.sync if m % 2 == 0 else nc.scalar
                    q_.dma_start(
                        out=outT_d.ap()[128 * m:128 * m + 128, cols], in_=fs)

            # ---------------- schedule ----------------
            v_proj(0, 4)
            qk_proj_half(wq_sb, qT, 0, 0, "q")
            qk_proj_half(wk_sb, kT, 0, 0, "k")
            attention(0, 0)
            qk_proj_half(wq_sb, qT, 0, 1, "q")
            qk_proj_half(wk_sb, kT, 0, 1, "k")
            v_proj(4, 8)
            qk_proj_half(wq_sb, qT, 1, 0, "q")
            qk_proj_half(wk_sb, kT, 1, 0, "k")
            attention(0, 1)
            qk_proj_half(wq_sb, qT, 1, 1, "q")
            qk_proj_half(wk_sb, kT, 1, 1, "k")
            v_proj(8, 12)
            attention(1, 0)
            v_proj(12, 16)
            attention(1, 1)
            out_proj(0)
            attention(2, 0)
            attention(2, 1)
            out_proj(1)
            attention(3, 0)
            out_proj(2)
            attention(3, 1)
            out_proj(3)

    nc.compile()
    return nc


def _consts():
    i = np.arange(32)
    theta = 1.0 / (10000.0 ** (2.0 * i / 64))
    ang = np.outer(np.arange(T, dtype=np.float64), theta)
    p = np.arange(128)
    cos2 = np.cos(ang[:, p % 32]).T
    sgn = np.where((p % 64) < 32, -1.0, 1.0)
    sin2s = (np.sin(ang[:, p % 32]) * sgn).T
    # pre-shuffled sin so the kernel can multiply BEFORE the partition
    # shuffle: shuf(q * sin2p)[p] = q[p^32] * sin2s[p]
    sin2p = sin2s[p ^ 32]
    r, c = np.meshgrid(np.arange(128), np.arange(128), indexing="ij")
    tri = (r <= c).astype(np.float64)
    tri2 = np.broadcast_to(tri[:, None, :], (128, 2, 128)).reshape(128, 256)
    cos2 = np.ascontiguousarray(cos2).astype(NPBF16)
    sin2p = np.ascontiguousarray(sin2p).astype(NPBF16)
    tri2 = np.ascontiguousarray(tri2).astype(NPBF16)
    return cos2, sin2p, tri2


def kernel(x, Wq, Wk, Wv, Wo, _trace=False):
    x = np.asarray(x, dtype=np.float32)
    Wq = np.asarray(Wq, dtype=np.float32)
    Wk = np.asarray(Wk, dtype=np.float32)
    Wv = np.asarray(Wv, dtype=np.float32)
    Wo = np.asarray(Wo, dtype=np.float32)

    if "nc" not in _NC_CACHE:
        _NC_CACHE["nc"] = _build_nc()
    nc = _NC_CACHE["nc"]

    cos2, sin2p, tri2 = _consts()
    xTs = [np.ascontiguousarray(x[b].T).astype(NPBF16) for b in range(B)]
    WqT, WkT, WvT, WoT = Wq.T, Wk.T, Wv.T, Wo.T

    in_maps = []
    for c in range(8):
        b, g = c // HG, c % HG
        cs = slice(EC * g, EC * g + EC)
        in_maps.append({
            "xT": xTs[b],
            "wqT": np.ascontiguousarray(WqT[:, cs]).astype(NPBF16),
            "wkT": np.ascontiguousarray(WkT[:, cs]).astype(NPBF16),
            "wvT": np.ascontiguousarray(WvT[:, cs]).astype(NPBF16),
            "woT": np.ascontiguousarray(WoT[cs, :]).astype(NPBF16),
            "cos2": cos2, "sin2p": sin2p, "tri2": tri2,
        })

    kw = {}
    if _trace:
        kw = dict(trace=True, trace_cores=list(range(8)))
    res = run_bass_kernel_spmd(nc, in_maps, core_ids=list(range(8)), **kw)

    out = np.zeros((B, T, D), np.float32)
    for c in range(8):
        out[c // HG] += res.results[c]["outT"].astype(np.float32).T
    if _trace:
        return out, res
    return out
